# revision 55
# baseline (speedup 1.0000x reference)
"""Trainium2 Bass kernel for nn_DigitConvolutionalModel.

Model: x[B,784] -> conv3x3(valid, 28x28->26x26) -> flatten -> Linear(676,256)
       -> relu -> Linear(256,10).

The conv is linear, so it is folded into the first Linear on the host:
  h_pre = x @ W1eff + b1,  W1eff[784,256] = C @ W1.T  (C = conv as matrix)
leaving a plain 2-layer MLP for the device:
  out = relu(x @ W1eff + b1) @ W2.T + b2

Sharding: pure data parallelism over the batch dim across 8 NeuronCores
(8192 samples/core); weights replicated. Compute in bf16 with fp32 PSUM
accumulation. x is transposed on the host so the contraction dim (784,
zero-padded to 7*128) lands on SBUF partitions.
"""

import sys

if "/opt/trn_rl_repo" not in sys.path:
    sys.path.insert(0, "/opt/trn_rl_repo")

import ml_dtypes
import numpy as np

B = 65536
NCORES = 8
BC = B // NCORES  # 8192 samples per core
P = 128
KC = 7            # contraction chunks of 128 (784 zero-padded to 896)
NF1 = 256         # layer-1 output features (2 halves of 128)
NO = 10           # logits
NB = 512          # batch columns per matmul group (one PSUM bank, fp32)
NGRP = BC // NB   # 16 groups per core

_PROG = None


def _build_program():
    import concourse.tile as tile
    from concourse import bacc, mybir

    bf16 = mybir.dt.bfloat16
    f32 = mybir.dt.float32

    nc = bacc.Bacc("TRN2", target_bir_lowering=False, debug=False,
                   num_devices=NCORES)
    xt = nc.dram_tensor("xt", [P, NGRP, KC, NB], bf16,
                        kind="ExternalInput").ap()
    w1 = nc.dram_tensor("w1", [P, KC, NF1], bf16, kind="ExternalInput").ap()
    w2 = nc.dram_tensor("w2", [P, 2, NO], bf16, kind="ExternalInput").ap()
    b1 = nc.dram_tensor("b1", [P, 2], f32, kind="ExternalInput").ap()
    b2 = nc.dram_tensor("b2", [NO, 1], f32, kind="ExternalInput").ap()
    out = nc.dram_tensor("out", [NO, BC], f32, kind="ExternalOutput").ap()

    with tile.TileContext(nc) as tc:
        with (
            tc.tile_pool(name="singles", bufs=1) as singles,
            tc.tile_pool(name="xp", bufs=6) as xp,
            tc.tile_pool(name="hp", bufs=10) as hp,
            tc.tile_pool(name="op", bufs=8) as op,
            tc.tile_pool(name="ps1", bufs=4, space="PSUM") as ps1p,
            tc.tile_pool(name="ps2", bufs=4, space="PSUM") as ps2p,
        ):
            # PE warm-up: dummy matmuls on a zeroed tile keep the PE busy
            # through the initial DMA wait so HAM un-throttles (K=8/8)
            # before the first real matmul.
            wsb = singles.tile([P, P], bf16)
            nc.vector.memset(wsb, 0.0)
            wp = ps2p.tile([32, P], f32, tag="ps2", name="warm")
            NWARM = 48
            for i in range(NWARM):
                nc.tensor.matmul(wp, wsb[:, :32], wsb,
                                 start=(i == 0), stop=(i == NWARM - 1))

            # setup DMAs on the scalar queue so the sync queue can start
            # dispatching x loads immediately; w1/b1 first — they gate the
            # first matmul / first relu
            w1sb = singles.tile([P, KC, NF1], bf16)
            # first chunks land first: the opening matmuls gate on w1[:, :2]
            # (128KB) instead of the whole 448KB weight transfer
            nc.scalar.dma_start(out=w1sb[:, :2], in_=w1[:, :2])
            nc.scalar.dma_start(out=w1sb[:, 2:], in_=w1[:, 2:])
            b1sb = singles.tile([P, 2], f32)
            nc.scalar.dma_start(out=b1sb, in_=b1)
            b2sb = singles.tile([NO, 1], f32)
            nc.scalar.dma_start(out=b2sb, in_=b2)
            w2sb = singles.tile([P, 2, NO], bf16)
            nc.scalar.dma_start(out=w2sb, in_=w2)

            def layer2(hs, g):
                # layer-2 for group g, emitted one group late so the PE
                # never head-of-line-blocks on the relu ACTs
                gs = slice(g * NB, (g + 1) * NB)
                ps2 = ps2p.tile([NO, NB], f32, tag="ps2", name=f"ps2_{g}")
                for m in range(2):
                    nc.tensor.matmul(ps2, w2sb[:, m, :], hs[m],
                                     start=(m == 0), stop=(m == 1))
                osb = op.tile([NO, NB], f32, tag="o", name=f"o_{g}")
                nc.vector.tensor_scalar_add(osb, ps2, b2sb)
                nc.sync.dma_start(out=out[:, gs], in_=osb)

            pend = []
            for g in range(NGRP - 1):
                xg = xp.tile([P, KC, NB], bf16, tag="x", name=f"x_{g}")
                if g <= 1:
                    # groups 0/1 run during the bandwidth-contended startup:
                    # split their loads in two so the matmuls only wait for
                    # half the transfer
                    nc.sync.dma_start(out=xg[:, :, :NB // 2],
                                      in_=xt[:, g, :, :NB // 2])
                    nc.sync.dma_start(out=xg[:, :, NB // 2:],
                                      in_=xt[:, g, :, NB // 2:])
                    pss = [ps1p.tile([P, NB], f32, tag="ps1",
                                     name=f"ps1_{g}_{m}") for m in range(2)]
                    for h2 in range(2):
                        hsl = slice(h2 * NB // 2, (h2 + 1) * NB // 2)
                        for k in range(KC):
                            for m in range(2):
                                nc.tensor.matmul(
                                    pss[m][:, hsl],
                                    w1sb[:, k, m * P:(m + 1) * P],
                                    xg[:, k, hsl],
                                    start=(k == 0),
                                    stop=(k == KC - 1),
                                )
                    hs = []
                    for m in range(2):
                        h = hp.tile([P, NB], bf16, tag="h", name=f"h_{g}_{m}")
                        nc.scalar.activation(
                            h, pss[m], mybir.ActivationFunctionType.Relu,
                            bias=b1sb[:, m:m + 1],
                        )
                        hs.append(h)
                    if g == 0:
                        # group 0's layer-2 goes right here: the PE would
                        # otherwise idle waiting for x1's transfer anyway
                        layer2(hs, g)
                    else:
                        pend.append((hs, g))
                    continue
                nc.sync.dma_start(out=xg, in_=xt[:, g])

                pss = [ps1p.tile([P, NB], f32, tag="ps1", name=f"ps1_{g}_{m}")
                       for m in range(2)]
                for k in range(KC):
                    for m in range(2):
                        nc.tensor.matmul(
                            pss[m],
                            w1sb[:, k, m * P:(m + 1) * P],
                            xg[:, k, :],
                            start=(k == 0),
                            stop=(k == KC - 1),
                        )

                if len(pend) == 2:
                    layer2(*pend.pop(0))

                hs = []
                for m in range(2):
                    h = hp.tile([P, NB], bf16, tag="h", name=f"h_{g}_{m}")
                    nc.scalar.activation(
                        h, pss[m], mybir.ActivationFunctionType.Relu,
                        bias=b1sb[:, m:m + 1],
                    )
                    hs.append(h)
                pend.append((hs, g))

            # last group runs as two 256-column halves with the relu split
            # across the scalar and vector engines — shortens the final
            # dependency chain (L1 -> relu -> L2 -> bias -> store) that sits
            # fully exposed in the kernel tail
            gl = NGRP - 1
            NH = NB // 2
            xg = xp.tile([P, KC, NB], bf16, tag="x", name=f"x_{gl}")
            nc.sync.dma_start(out=xg, in_=xt[:, gl])
            for sub in range(2):
                cs = slice(sub * NH, (sub + 1) * NH)
                pss = [ps1p.tile([P, NH], f32, tag="ps1",
                                 name=f"ps1_{gl}_{sub}_{m}") for m in range(2)]
                for k in range(KC):
                    for m in range(2):
                        nc.tensor.matmul(
                            pss[m],
                            w1sb[:, k, m * P:(m + 1) * P],
                            xg[:, k, cs],
                            start=(k == 0),
                            stop=(k == KC - 1),
                        )
                if pend:
                    layer2(*pend.pop(0))
                h0 = hp.tile([P, NH], bf16, tag="h", name=f"h_{gl}_{sub}_0")
                nc.scalar.activation(h0, pss[0],
                                     mybir.ActivationFunctionType.Relu,
                                     bias=b1sb[:, 0:1])
                h1 = hp.tile([P, NH], bf16, tag="h", name=f"h_{gl}_{sub}_1")
                nc.vector.tensor_scalar(h1, pss[1], b1sb[:, 1:2], 0.0,
                                        mybir.AluOpType.add,
                                        mybir.AluOpType.max)
                ps2 = ps2p.tile([NO, NH], f32, tag="ps2",
                                name=f"ps2_{gl}_{sub}")
                for m, h in enumerate((h0, h1)):
                    nc.tensor.matmul(ps2, w2sb[:, m, :], h,
                                     start=(m == 0), stop=(m == 1))
                osb = op.tile([NO, NH], f32, tag="o", name=f"o_{gl}_{sub}")
                nc.vector.tensor_scalar_add(osb, ps2, b2sb)
                nc.sync.dma_start(
                    out=out[:, gl * NB + sub * NH:gl * NB + (sub + 1) * NH],
                    in_=osb)

    nc.compile()
    return nc


def _fold_weights(conv_w, W1):
    """W1eff[784,256] such that x @ W1eff == flatten(conv(x)) @ W1.T."""
    cw = conv_w.astype(np.float64)
    W1r = W1.astype(np.float64).reshape(NF1, 26, 26).transpose(1, 2, 0)
    W1eff = np.zeros((28, 28, NF1), np.float64)
    for dr in range(3):
        for dc in range(3):
            W1eff[dr:dr + 26, dc:dc + 26, :] += cw[dr, dc] * W1r
    return W1eff.reshape(784, NF1)


def _prep_inputs(x, conv_w, W1, b1, W2, b2):
    bf16 = ml_dtypes.bfloat16
    W1eff = _fold_weights(conv_w, W1)
    w1p = np.zeros((KC * P, NF1), np.float64)
    w1p[:784] = W1eff
    w1p = np.ascontiguousarray(
        w1p.reshape(KC, P, NF1).transpose(1, 0, 2)).astype(bf16)  # [P, KC, NF1]
    w2p = np.ascontiguousarray(
        W2.T.astype(np.float32).reshape(2, P, NO).transpose(1, 0, 2)).astype(bf16)
    b1p = np.ascontiguousarray(b1.astype(np.float32).reshape(2, P).T)  # [P, 2]
    b2p = b2.astype(np.float32).reshape(NO, 1)

    in_maps = []
    for c in range(NCORES):
        xc = np.zeros((KC * P, BC), bf16)
        xcT = np.ascontiguousarray(x[c * BC:(c + 1) * BC].T)  # [784, BC] f32
        xc[:784] = xcT.astype(bf16)
        # device layout [P, NGRP, KC, NB]: one batch group is a single DMA of
        # KC*NB contiguous bytes per partition
        xdev = xc.reshape(KC, P, NGRP, NB).transpose(1, 2, 0, 3)
        in_maps.append({
            "xt": np.ascontiguousarray(xdev),
            "w1": w1p, "w2": w2p, "b1": b1p, "b2": b2p,
        })
    return in_maps


def kernel(x, conv_w, W1, b1, W2, b2, _trace=False, _trace_kwargs=None):
    global _PROG
    from concourse import bass_utils

    x = np.asarray(x, dtype=np.float32)
    conv_w = np.asarray(conv_w, dtype=np.float32)
    W1 = np.asarray(W1, dtype=np.float32)
    b1 = np.asarray(b1, dtype=np.float32)
    W2 = np.asarray(W2, dtype=np.float32)
    b2 = np.asarray(b2, dtype=np.float32)
    assert x.shape == (B, 784), x.shape

    if _PROG is None:
        _PROG = _build_program()

    in_maps = _prep_inputs(x, conv_w, W1, b1, W2, b2)
    kwargs = dict(_trace_kwargs or {})
    res = bass_utils.run_bass_kernel_spmd(
        _PROG, in_maps, core_ids=list(range(NCORES)), trace=_trace, **kwargs)

    out = np.empty((B, NO), np.float32)
    for c in range(NCORES):
        out[c * BC:(c + 1) * BC] = res.results[c]["out"].T
    if _trace:
        return out, res
    return out
